# revision 2
# baseline (speedup 1.0000x reference)
"""Trainium2 Bass kernel for attention-score softmax.

Computes, for input_sec [B=8, S=8192, D=1024], state [B, D], w [D], b [1]:
    energy[b, s] = dot(tanh(input_sec[b, s, :] + state[b, :]), w) + b
    out[b, :]    = softmax(energy[b, :], axis=-1)

Sharding: data-parallel over batch, one batch element per NeuronCore (8 cores).

Per-core dataflow on transposed input xT [D, S] in fp16 (host-side cast —
halves DMA traffic; this kernel is memory-bound):
  - x arrives in column pieces via two parallel HWDGE rings (sync + scalar
    queues) so per-piece completion stalls overlap across rings.
  - tanh is split across TWO engines to break the ScalarE 1-elem/cycle wall:
      * ACT (ScalarE): 6 of 8 d-blocks, exact tanh, bias=state fused.
      * DVE (VectorE): 2 d-blocks via a deg-9 odd polynomial on the clamped
        input (max err 4.8e-3), evaluated in 2 custom 8-stage DVE ops;
        the bias-add + upper clamp runs as a tensor_scalar on GpSimd (Pool),
        which is otherwise idle.  f(u) = v*(L*t^2+a1*t+b1)*(t^2+a2*t+b2),
        v = clamp(u, +-B), t = v*v.
  - TensorE: energy = w . t accumulated over all pieces into one PSUM tile
    [16, 512]; sequence chunk j lands on PSUM partition j via block-diagonal
    weight columns.
  - ScalarE: p = exp(energy) with fused row sums; TensorE ones-matmul reduces
    and broadcasts the total; VectorE reciprocal + scale; DMA out.
"""

import os
from contextlib import ExitStack

import numpy as np

import concourse.bacc as bacc
import concourse.tile as tile
from concourse import mybir
from concourse.bass_utils import run_bass_kernel_spmd

B, S, D = 8, 8192, 1024
NB_D = D // 128          # 8 d-blocks
N_CHUNK = S // 512       # 16 sequence chunks of 512

# --- DVE tanh approximation constants (fit: max err 4.8e-3 on |u|<=9) -------
TANH_B = 2.848135051824187       # clamp bound
TANH_L = 0.0002268581482379952   # leading coeff of q1
TANH_A1 = -0.004724477388143275
TANH_B1 = 0.028162570473750825
TANH_A2 = -3.3548299414719067
TANH_B2 = 34.733766917451845

DVE_BLOCKS = (2, 5)

# Global piece schedule: (block, col0, width, queue) in issue order.
# Queue: 0 = sync (qSPDynamicHW), 1 = scalar (qActDynamicHW).
# DVE-consumed pieces (blocks 2,5) are scheduled early so the 3-stage
# Pool->DVE->DVE chain drains before the ACT stream finishes.
PIECE_SCHED = [
    (0, 0,    2048, 0),
    (2, 0,    2048, 1),
    (1, 0,    2048, 0),
    (2, 2048, 2048, 1),
    (0, 2048, 2048, 0),
    (3, 0,    4096, 1),
    (2, 4096, 2048, 0),
    (1, 2048, 2048, 1),
    (2, 6144, 2048, 0),
    (0, 4096, 4096, 1),
    (5, 0,    2048, 0),
    (4, 0,    4096, 1),
    (5, 2048, 2048, 0),
    (1, 4096, 4096, 1),
    (5, 4096, 2048, 0),
    (3, 4096, 4096, 1),
    (5, 6144, 2048, 0),
    (4, 4096, 4096, 1),
    (6, 0,    4096, 0),
    (7, 0,    4096, 1),
    (6, 4096, 4096, 0),
    (7, 4096, 3072, 1),
    (7, 7168, 1024, 0),
]

_compiled = {}
last_result = None  # BassKernelResults of the most recent run (for test harness)


# --- custom DVE op registration --------------------------------------------
def _register_dve_ops():
    """Register the two custom DVE tanh ops in concourse.dve_ops (idempotent).

    P1: v = max(in0, s0); t = v*v; out = ((L*t + a1)*t + b1) * v
        [s0=-B, C3(in1,[P,1])=L, s1=a1, imm2=b1]
    P2: v = max(in0, s0); t = v*v; out = ((t + a2)*t + b2) * in1
        [s0=-B, s1=a2, imm2=b2, in1 = g1 tensor]
    (the upper clamp min(u, B) is done by the preceding tensor_scalar)
    """
    import concourse.dve_ops as dve_ops
    from concourse.dve_spec import (
        Spec, Src0, Src1, C0, C1, C2, C3, sq, maxx,
        lower, _spill_c3_to_src1,
    )
    from concourse.dve_uop import DveOpSpec

    if "ATTN_TANH_P1" in dve_ops._SUB_OPCODE_FOR_NAME:
        by_name = {op.name: op for op in dve_ops.OPS}
        return by_name["ATTN_TANH_P1"], by_name["ATTN_TANH_P2"]

    def ref_p1(in0, in1, s0, s1, imm2):
        v = np.maximum(in0.astype(np.float32), np.float32(s0))
        t = v * v
        return (((in1.astype(np.float32) * t + np.float32(s1)) * t
                 + np.float32(imm2)) * v).astype(np.float32)

    def ref_p2(in0, in1, s0, s1, imm2):
        v = np.maximum(in0.astype(np.float32), np.float32(s0))
        t = v * v
        return (((t + np.float32(s1)) * t + np.float32(imm2))
                * in1.astype(np.float32)).astype(np.float32)

    _v1 = maxx(Src0, C0)
    _t1 = sq(_v1)
    body1 = _spill_c3_to_src1(((C3 * _t1 + C1) * _t1 + C2) * _v1)
    spec1 = Spec(body=body1, reference=ref_p1)

    _v2 = maxx(Src0, C0)
    _t2 = sq(_v2)
    body2 = ((_t2 + C1) * _t2 + C2) * Src1
    spec2 = Spec(body=body2, reference=ref_p2)

    ops = []
    for name, spec in [("ATTN_TANH_P1", spec1), ("ATTN_TANH_P2", spec2)]:
        opcode = dve_ops._CUSTOM_DVE_ROW_BASE + len(dve_ops.OPS)
        assert opcode < 0x20
        shas = {}
        for ver in ("v3", "v4"):
            s = DveOpSpec(name=name, opcode=opcode,
                          uops=lower(spec, ver=ver), rd1_en=True)
            shas[ver] = s.sha(ver)
        op = dve_ops.DveOp(name, spec, subdim=False, uops_sha=shas)
        dve_ops.OPS.append(op)
        dve_ops.CUSTOM_DVE_SPECS[name] = spec
        dve_ops._SUB_OPCODE_FOR_NAME[name] = opcode
        ops.append(op)
    return ops[0], ops[1]


def _build():
    P1, P2 = _register_dve_ops()
    xdt = mybir.dt.float16
    f32 = mybir.dt.float32

    nc = bacc.Bacc()
    xT = nc.declare_dram_parameter("xT", [D, S], xdt, isOutput=False)
    state_cols = nc.declare_dram_parameter("state_cols", [128, NB_D], f32,
                                           isOutput=False)
    w_blk = nc.declare_dram_parameter("w_blk", [NB_D, 128, 16 * 16], xdt,
                                      isOutput=False)
    out_ext = nc.declare_dram_parameter("out", [S], f32, isOutput=True)

    dma_q = {}

    with tile.TileContext(nc) as tc, ExitStack() as ctx:
        consts = ctx.enter_context(tc.tile_pool(name="consts", bufs=1))
        tpool = ctx.enter_context(tc.tile_pool(name="t", bufs=1))
        g1pool = ctx.enter_context(tc.tile_pool(name="g1", bufs=3))
        tailp = ctx.enter_context(tc.tile_pool(name="tail", bufs=1))
        psum = ctx.enter_context(tc.tile_pool(name="psum", bufs=2, space="PSUM"))

        # Dummy activation with no data deps: pulls the ACT_TABLE_LOAD
        # (~1.3 us, exp_and_others covers Tanh+Exp) into the preamble.
        warm = consts.tile([128, 1], f32)
        nc.vector.memset(warm, 0.0)
        nc.scalar.activation(out=warm, in_=warm,
                             func=mybir.ActivationFunctionType.Tanh)

        state_sb = consts.tile([128, NB_D], f32)
        nc.gpsimd.dma_start(out=state_sb, in_=state_cols[:])
        w_sb = consts.tile([128, NB_D, 256], xdt)
        nc.gpsimd.dma_start(out=w_sb, in_=w_blk[:].rearrange("i p c -> p i c"))

        lconst = consts.tile([128, 1], f32)
        nc.vector.memset(lconst, TANH_L)
        ones_sb = consts.tile([128, 16], f32)
        nc.vector.memset(ones_sb, 1.0)
        sums_sb = consts.tile([128, 1], f32)
        nc.vector.memset(sums_sb, 0.0)

        # piece tiles (resident; x for a DVE piece is overwritten in place:
        # ts: x <- min(x + state, B); p2: x <- tanh_approx)
        tiles = {}
        for k, (i, c0, w, q) in enumerate(PIECE_SCHED):
            t_t = tpool.tile([128, w], xdt, tag=f"t{k}", name=f"t{k}")
            tiles[k] = t_t

        # DMA issues, alternating between the two HWDGE rings
        for k, (i, c0, w, q) in enumerate(PIECE_SCHED):
            eng = nc.sync if q == 0 else nc.scalar
            eng.dma_start(
                out=tiles[k], in_=xT[:][128 * i:128 * (i + 1), c0:c0 + w],
            )

        # compute: ACT tanh for ACT blocks; Pool ts + DVE p1/p2 for DVE blocks
        for k, (i, c0, w, q) in enumerate(PIECE_SCHED):
            t_t = tiles[k]
            if i in DVE_BLOCKS:
                nc.gpsimd.tensor_scalar(
                    out=t_t, in0=t_t,
                    scalar1=state_sb[:, i:i + 1], scalar2=TANH_B,
                    op0=mybir.AluOpType.add, op1=mybir.AluOpType.min,
                )
                g1 = g1pool.tile([128, w], xdt, tag="g1", name=f"g1_{k}")
                nc.vector._custom_dve(
                    P1, out=g1, in0=t_t, in1=lconst,
                    s0=-TANH_B, s1=TANH_A1, imm2=TANH_B1,
                )
                nc.vector._custom_dve(
                    P2, out=t_t, in0=t_t, in1=g1,
                    s0=-TANH_B, s1=TANH_A2, imm2=TANH_B2,
                )
            else:
                nc.scalar.activation(
                    out=t_t, in_=t_t,
                    func=mybir.ActivationFunctionType.Tanh,
                    bias=state_sb[:, i:i + 1], scale=1.0,
                )

        energy_ps = psum.tile([16, 512], f32)
        n_mm = 0
        n_total = sum(w // 512 for (_, _, w, _) in PIECE_SCHED)
        for k, (i, c0, w, q) in enumerate(PIECE_SCHED):
            t_t = tiles[k]
            for c in range(c0 // 512, (c0 + w) // 512):
                off = 512 * c - c0
                n_mm += 1
                nc.tensor.matmul(
                    energy_ps[:],
                    lhsT=w_sb[:, i, 16 * c:16 * (c + 1)],
                    rhs=t_t[:, off:off + 512],
                    start=(n_mm == 1),
                    stop=(n_mm == n_total),
                )

        # softmax tail (softmax max-subtraction is skipped: |energy| <= ||w||_1
        # ~ 26, exp is safely in fp32 range; the bias b never affects softmax)
        p_sb = tailp.tile([16, 512], f32)
        nc.scalar.activation(
            out=p_sb, in_=energy_ps[:],
            func=mybir.ActivationFunctionType.Exp,
            bias=0.0, scale=1.0,
            accum_out=sums_sb[0:16, :],
        )
        sum_ps = psum.tile([16, 1], f32)
        nc.tensor.matmul(sum_ps[:], lhsT=ones_sb, rhs=sums_sb,
                         start=True, stop=True)
        inv_sb = tailp.tile([16, 1], f32)
        nc.vector.reciprocal(out=inv_sb, in_=sum_ps[:])
        out_sb = tailp.tile([16, 512], f32)
        nc.vector.tensor_scalar_mul(out=out_sb, in0=p_sb, scalar1=inv_sb)
        nc.sync.dma_start(
            out=out_ext[:].rearrange("(p f) -> p f", p=16), in_=out_sb,
        )

    nc.finalize()
    return nc


def _get_nc():
    if "nc" not in _compiled:
        _compiled["nc"] = _build()
    return _compiled["nc"]


def kernel(input_sec, state, w, b=None, **_unused):
    np_xdt = np.float16
    nc = _get_nc()

    # host-side layout prep (single-pass strided read + cast + pack)
    xT_all = np.asarray(input_sec).transpose(0, 2, 1).astype(np_xdt)  # [B, D, S]
    state_cols_all = np.ascontiguousarray(
        np.asarray(state, np.float32).reshape(B, NB_D, 128).transpose(0, 2, 1)
    )                                                          # [B, 128, NB_D]
    w_grid = np.asarray(w, np.float32).reshape(NB_D, 128)
    w_blk = np.zeros((NB_D, 128, 16, 16), np.float32)
    for j in range(16):
        w_blk[:, :, j, j] = w_grid
    w_blk = w_blk.reshape(NB_D, 128, 256).astype(np_xdt)

    in_maps = [
        {
            "xT": xT_all[c],
            "state_cols": state_cols_all[c],
            "w_blk": w_blk,
        }
        for c in range(B)
    ]
    trace = bool(int(os.environ.get("ATTN_KERNEL_TRACE", "0")))
    res = run_bass_kernel_spmd(nc, in_maps, core_ids=list(range(B)),
                               trace=trace)
    global last_result
    last_result = res
    out = np.stack([res.results[c]["out"] for c in range(B)], axis=0)
    return out.astype(np.float32)


# revision 4
# speedup vs baseline: 3.6502x; 3.6502x over previous
"""Trainium2 Bass kernel for attention-score softmax.

Computes, for input_sec [B=8, S=8192, D=1024], state [B, D], w [D], b [1]:
    energy[b, s] = dot(tanh(input_sec[b, s, :] + state[b, :]), w) + b
    out[b, :]    = softmax(energy[b, :], axis=-1)

Sharding: data-parallel over batch, one batch element per NeuronCore (8 cores).

Per-core dataflow on transposed input xT [D, S] in fp16 (host-side cast —
halves DMA traffic; this kernel is memory-bound):
  - x arrives in column pieces via two parallel HWDGE rings (sync + scalar
    queues) so per-piece completion stalls overlap across rings.
  - tanh is split across TWO engines to break the ScalarE 1-elem/cycle wall:
      * ACT (ScalarE): 6 of 8 d-blocks, exact tanh, bias=state fused.
      * DVE (VectorE): 2 d-blocks via a deg-9 odd polynomial on the clamped
        input (max err 4.8e-3), evaluated in 2 custom 8-stage DVE ops;
        the bias-add + upper clamp runs as a tensor_scalar on GpSimd (Pool),
        which is otherwise idle.  f(u) = v*(L*t^2+a1*t+b1)*(t^2+a2*t+b2),
        v = clamp(u, +-B), t = v*v.
  - TensorE: energy = w . t accumulated over all pieces into one PSUM tile
    [16, 512]; sequence chunk j lands on PSUM partition j via block-diagonal
    weight columns.
  - ScalarE: p = exp(energy) with fused row sums; TensorE ones-matmul reduces
    and broadcasts the total; VectorE reciprocal + scale; DMA out.
"""

import os
from contextlib import ExitStack

import numpy as np

import concourse.bacc as bacc
import concourse.tile as tile
from concourse import mybir
from concourse.bass_utils import run_bass_kernel_spmd

B, S, D = 8, 8192, 1024
NB_D = D // 128          # 8 d-blocks
N_CHUNK = S // 512       # 16 sequence chunks of 512

# --- DVE tanh approximation constants (fit: max err 4.8e-3 on |u|<=9) -------
TANH_B = 2.848135051824187       # clamp bound
TANH_L = 0.0002268581482379952   # leading coeff of q1
TANH_A1 = -0.004724477388143275
TANH_B1 = 0.028162570473750825
TANH_A2 = -3.3548299414719067
TANH_B2 = 34.733766917451845

# (block, piece col range) pairs consumed by the DVE path; everything else ACT.
DVE_PIECES = {(2, 0), (2, 4096), (5, 0)}   # 1.5 of 8 d-blocks

# Global piece schedule: (block, col0, width, queue) in issue order.
# Queue: 0 = sync (qSPDynamicHW ring), 1 = gpsimd (SWDGE ring) — two DMA
# rings processing concurrently so per-piece completion stalls overlap.
# The scalar/vector queues carry no DMA issues: issues can block on ring
# space and would stall the compute behind them in the queue.
# DVE-consumed pieces are scheduled early so the 3-pass chain drains
# before the ACT stream finishes.
PIECE_SCHED = [
    (0, 0,    2048, 0),
    (2, 0,    4096, 1),
    (1, 0,    2048, 0),
    (3, 0,    4096, 1),
    (0, 2048, 2048, 0),
    (2, 4096, 4096, 1),
    (1, 2048, 2048, 0),
    (4, 0,    4096, 1),
    (0, 4096, 4096, 0),
    (5, 0,    4096, 1),
    (1, 4096, 4096, 0),
    (3, 4096, 4096, 1),
    (5, 4096, 4096, 0),
    (4, 4096, 4096, 1),
    (6, 0,    4096, 0),
    (7, 0,    4096, 1),
    (6, 4096, 4096, 0),
    (7, 4096, 3072, 1),
    (7, 7168, 1024, 0),
]

_compiled = {}
last_result = None  # BassKernelResults of the most recent run (for test harness)


# --- custom DVE op registration --------------------------------------------
def _register_dve_ops():
    """Register the two custom DVE tanh ops in concourse.dve_ops (idempotent).

    P1: v = max(in0, s0); t = v*v; out = ((L*t + a1)*t + b1) * v
        [s0=-B, C3(in1,[P,1])=L, s1=a1, imm2=b1]
    P2: v = max(in0, s0); t = v*v; out = ((t + a2)*t + b2) * in1
        [s0=-B, s1=a2, imm2=b2, in1 = g1 tensor]
    (the upper clamp min(u, B) is done by the preceding tensor_scalar)
    """
    import concourse.dve_ops as dve_ops
    from concourse.dve_spec import (
        Spec, Src0, Src1, C0, C1, C2, C3, sq, maxx,
        lower, _spill_c3_to_src1,
    )
    from concourse.dve_uop import DveOpSpec

    if "ATTN_TANH_P1" in dve_ops._SUB_OPCODE_FOR_NAME:
        by_name = {op.name: op for op in dve_ops.OPS}
        return by_name["ATTN_TANH_P1"], by_name["ATTN_TANH_P2"]

    def ref_p1(in0, in1, s0, s1, imm2):
        v = np.maximum(in0.astype(np.float32), np.float32(s0))
        t = v * v
        return (((in1.astype(np.float32) * t + np.float32(s1)) * t
                 + np.float32(imm2)) * v).astype(np.float32)

    def ref_p2(in0, in1, s0, s1, imm2):
        v = np.maximum(in0.astype(np.float32), np.float32(s0))
        t = v * v
        return (((t + np.float32(s1)) * t + np.float32(imm2))
                * in1.astype(np.float32)).astype(np.float32)

    _v1 = maxx(Src0, C0)
    _t1 = sq(_v1)
    body1 = _spill_c3_to_src1(((C3 * _t1 + C1) * _t1 + C2) * _v1)
    spec1 = Spec(body=body1, reference=ref_p1)

    _v2 = maxx(Src0, C0)
    _t2 = sq(_v2)
    body2 = ((_t2 + C1) * _t2 + C2) * Src1
    spec2 = Spec(body=body2, reference=ref_p2)

    ops = []
    for name, spec in [("ATTN_TANH_P1", spec1), ("ATTN_TANH_P2", spec2)]:
        opcode = dve_ops._CUSTOM_DVE_ROW_BASE + len(dve_ops.OPS)
        assert opcode < 0x20
        shas = {}
        for ver in ("v3", "v4"):
            s = DveOpSpec(name=name, opcode=opcode,
                          uops=lower(spec, ver=ver), rd1_en=True)
            shas[ver] = s.sha(ver)
        op = dve_ops.DveOp(name, spec, subdim=False, uops_sha=shas)
        dve_ops.OPS.append(op)
        dve_ops.CUSTOM_DVE_SPECS[name] = spec
        dve_ops._SUB_OPCODE_FOR_NAME[name] = opcode
        ops.append(op)
    return ops[0], ops[1]


def _build():
    P1, P2 = _register_dve_ops()
    xdt = mybir.dt.float16
    f32 = mybir.dt.float32

    nc = bacc.Bacc()
    xT = nc.declare_dram_parameter("xT", [D, S], xdt, isOutput=False)
    state_cols = nc.declare_dram_parameter("state_cols", [128, NB_D], f32,
                                           isOutput=False)
    w_blk = nc.declare_dram_parameter("w_blk", [NB_D, 128, 16 * 16], xdt,
                                      isOutput=False)
    out_ext = nc.declare_dram_parameter("out", [S], f32, isOutput=True)

    dma_q = {}

    with tile.TileContext(nc) as tc, ExitStack() as ctx:
        consts = ctx.enter_context(tc.tile_pool(name="consts", bufs=1))
        tpool = ctx.enter_context(tc.tile_pool(name="t", bufs=1))
        g1pool = ctx.enter_context(tc.tile_pool(name="g1", bufs=3))
        tailp = ctx.enter_context(tc.tile_pool(name="tail", bufs=1))
        psum = ctx.enter_context(tc.tile_pool(name="psum", bufs=2, space="PSUM"))

        # Dummy activation with no data deps: pulls the ACT_TABLE_LOAD
        # (~1.3 us, exp_and_others covers Tanh+Exp) into the preamble.
        warm = consts.tile([128, 1], f32)
        nc.vector.memset(warm, 0.0)
        nc.scalar.activation(out=warm, in_=warm,
                             func=mybir.ActivationFunctionType.Tanh)

        state_sb = consts.tile([128, NB_D], f32)
        nc.gpsimd.dma_start(out=state_sb, in_=state_cols[:])
        w_sb = consts.tile([128, NB_D, 256], xdt)
        nc.gpsimd.dma_start(out=w_sb, in_=w_blk[:].rearrange("i p c -> p i c"))

        lconst = consts.tile([128, 1], f32)
        nc.vector.memset(lconst, TANH_L)
        ones_sb = consts.tile([128, 16], f32)
        nc.vector.memset(ones_sb, 1.0)
        sums_sb = consts.tile([128, 1], f32)
        nc.vector.memset(sums_sb, 0.0)

        # piece tiles (resident; x for a DVE piece is overwritten in place:
        # ts: x <- min(x + state, B); p2: x <- tanh_approx)
        tiles = {}
        for k, (i, c0, w, q) in enumerate(PIECE_SCHED):
            t_t = tpool.tile([128, w], xdt, tag=f"t{k}", name=f"t{k}")
            tiles[k] = t_t

        # DMA issues, alternating between the two DMA rings
        for k, (i, c0, w, q) in enumerate(PIECE_SCHED):
            eng = nc.sync if q == 0 else nc.gpsimd
            eng.dma_start(
                out=tiles[k], in_=xT[:][128 * i:128 * (i + 1), c0:c0 + w],
            )

        # compute: ACT tanh for ACT pieces; DVE ts + p1 + p2 for DVE pieces
        for k, (i, c0, w, q) in enumerate(PIECE_SCHED):
            t_t = tiles[k]
            if (i, c0) in DVE_PIECES:
                nc.vector.tensor_scalar(
                    out=t_t, in0=t_t,
                    scalar1=state_sb[:, i:i + 1], scalar2=TANH_B,
                    op0=mybir.AluOpType.add, op1=mybir.AluOpType.min,
                )
                g1 = g1pool.tile([128, w], xdt, tag="g1", name=f"g1_{k}")
                nc.vector._custom_dve(
                    P1, out=g1, in0=t_t, in1=lconst,
                    s0=-TANH_B, s1=TANH_A1, imm2=TANH_B1,
                )
                nc.vector._custom_dve(
                    P2, out=t_t, in0=t_t, in1=g1,
                    s0=-TANH_B, s1=TANH_A2, imm2=TANH_B2,
                )
            else:
                nc.scalar.activation(
                    out=t_t, in_=t_t,
                    func=mybir.ActivationFunctionType.Tanh,
                    bias=state_sb[:, i:i + 1], scale=1.0,
                )

        energy_ps = psum.tile([16, 512], f32)
        n_mm = 0
        n_total = sum(w // 512 for (_, _, w, _) in PIECE_SCHED)
        for k, (i, c0, w, q) in enumerate(PIECE_SCHED):
            t_t = tiles[k]
            for c in range(c0 // 512, (c0 + w) // 512):
                off = 512 * c - c0
                n_mm += 1
                nc.tensor.matmul(
                    energy_ps[:],
                    lhsT=w_sb[:, i, 16 * c:16 * (c + 1)],
                    rhs=t_t[:, off:off + 512],
                    start=(n_mm == 1),
                    stop=(n_mm == n_total),
                )

        # softmax tail (softmax max-subtraction is skipped: |energy| <= ||w||_1
        # ~ 26, exp is safely in fp32 range; the bias b never affects softmax)
        p_sb = tailp.tile([16, 512], f32)
        nc.scalar.activation(
            out=p_sb, in_=energy_ps[:],
            func=mybir.ActivationFunctionType.Exp,
            bias=0.0, scale=1.0,
            accum_out=sums_sb[0:16, :],
        )
        sum_ps = psum.tile([16, 1], f32)
        nc.tensor.matmul(sum_ps[:], lhsT=ones_sb, rhs=sums_sb,
                         start=True, stop=True)
        inv_sb = tailp.tile([16, 1], f32)
        nc.vector.reciprocal(out=inv_sb, in_=sum_ps[:])
        out_sb = tailp.tile([16, 512], f32)
        nc.vector.tensor_scalar_mul(out=out_sb, in0=p_sb, scalar1=inv_sb)
        nc.sync.dma_start(
            out=out_ext[:].rearrange("(p f) -> p f", p=16), in_=out_sb,
        )

    nc.finalize()
    return nc


def _get_nc():
    if "nc" not in _compiled:
        _compiled["nc"] = _build()
    return _compiled["nc"]


def kernel(input_sec, state, w, b=None, **_unused):
    np_xdt = np.float16
    nc = _get_nc()

    # host-side layout prep (single-pass strided read + cast + pack)
    xT_all = np.asarray(input_sec).transpose(0, 2, 1).astype(np_xdt)  # [B, D, S]
    state_cols_all = np.ascontiguousarray(
        np.asarray(state, np.float32).reshape(B, NB_D, 128).transpose(0, 2, 1)
    )                                                          # [B, 128, NB_D]
    w_grid = np.asarray(w, np.float32).reshape(NB_D, 128)
    w_blk = np.zeros((NB_D, 128, 16, 16), np.float32)
    for j in range(16):
        w_blk[:, :, j, j] = w_grid
    w_blk = w_blk.reshape(NB_D, 128, 256).astype(np_xdt)

    in_maps = [
        {
            "xT": xT_all[c],
            "state_cols": state_cols_all[c],
            "w_blk": w_blk,
        }
        for c in range(B)
    ]
    trace = bool(int(os.environ.get("ATTN_KERNEL_TRACE", "0")))
    res = run_bass_kernel_spmd(nc, in_maps, core_ids=list(range(B)),
                               trace=trace)
    global last_result
    last_result = res
    out = np.stack([res.results[c]["out"] for c in range(B)], axis=0)
    return out.astype(np.float32)


# revision 5
# speedup vs baseline: 3.9297x; 1.0766x over previous
"""Trainium2 Bass kernel for attention-score softmax.

Computes, for input_sec [B=8, S=8192, D=1024], state [B, D], w [D], b [1]:
    energy[b, s] = dot(tanh(input_sec[b, s, :] + state[b, :]), w) + b
    out[b, :]    = softmax(energy[b, :], axis=-1)

Sharding: data-parallel over batch, one batch element per NeuronCore (8 cores).

Per-core dataflow on transposed input xT [D, S] in fp16 (host-side cast —
halves DMA traffic; this kernel is memory-bound):
  - x arrives in column pieces via two parallel HWDGE rings (sync + scalar
    queues) so per-piece completion stalls overlap across rings.
  - tanh is split across TWO engines to break the ScalarE 1-elem/cycle wall:
      * ACT (ScalarE): 6 of 8 d-blocks, exact tanh, bias=state fused.
      * DVE (VectorE): 2 d-blocks via a deg-9 odd polynomial on the clamped
        input (max err 4.8e-3), evaluated in 2 custom 8-stage DVE ops;
        the bias-add + upper clamp runs as a tensor_scalar on GpSimd (Pool),
        which is otherwise idle.  f(u) = v*(L*t^2+a1*t+b1)*(t^2+a2*t+b2),
        v = clamp(u, +-B), t = v*v.
  - TensorE: energy = w . t accumulated over all pieces into one PSUM tile
    [16, 512]; sequence chunk j lands on PSUM partition j via block-diagonal
    weight columns.
  - ScalarE: p = exp(energy) with fused row sums; TensorE ones-matmul reduces
    and broadcasts the total; VectorE reciprocal + scale; DMA out.
"""

import os
from contextlib import ExitStack

import numpy as np

import concourse.bacc as bacc
import concourse.tile as tile
from concourse import mybir
from concourse.bass_utils import run_bass_kernel_spmd

B, S, D = 8, 8192, 1024
NB_D = D // 128          # 8 d-blocks
N_CHUNK = S // 512       # 16 sequence chunks of 512

# --- DVE tanh approximation constants (fit: max err 4.8e-3 on |u|<=9) -------
TANH_B = 2.848135051824187       # clamp bound
TANH_L = 0.0002268581482379952   # leading coeff of q1
TANH_A1 = -0.004724477388143275
TANH_B1 = 0.028162570473750825
TANH_A2 = -3.3548299414719067
TANH_B2 = 34.733766917451845

# (block, piece col range) pairs consumed by the DVE path; everything else ACT.
DVE_PIECES = {(2, 0), (2, 4096), (5, 0)}   # 1.5 of 8 d-blocks

# Global piece schedule: (block, col0, width, queue) in issue order.
# All x pieces ride ONE HWDGE ring (sync / qSPDynamicHW): a single
# sequential stream keeps the SDMA engines at the HBM ceiling — a second
# concurrent ring was measured to hurt HBM locality (~210 GB/s aggregate).
# DVE-consumed pieces are scheduled early so the 3-pass chain drains
# before the ACT stream finishes; first piece is small for a fast ACT start.
PIECE_SCHED = [
    (0, 0,    1024, 0),
    (2, 0,    4096, 0),
    (0, 1024, 1024, 0),
    (1, 0,    2048, 0),
    (0, 2048, 2048, 0),
    (2, 4096, 4096, 0),
    (1, 2048, 2048, 0),
    (0, 4096, 4096, 0),
    (5, 0,    4096, 0),
    (1, 4096, 4096, 0),
    (3, 0,    4096, 0),
    (3, 4096, 4096, 0),
    (4, 0,    4096, 0),
    (5, 4096, 4096, 0),
    (4, 4096, 4096, 0),
    (6, 0,    4096, 0),
    (6, 4096, 4096, 0),
    (7, 0,    4096, 0),
    (7, 4096, 3072, 0),
    (7, 7168, 1024, 0),
]

_compiled = {}
last_result = None  # BassKernelResults of the most recent run (for test harness)


# --- custom DVE op registration --------------------------------------------
def _register_dve_ops():
    """Register the two custom DVE tanh ops in concourse.dve_ops (idempotent).

    P1: v = max(in0, s0); t = v*v; out = ((L*t + a1)*t + b1) * v
        [s0=-B, C3(in1,[P,1])=L, s1=a1, imm2=b1]
    P2: v = max(in0, s0); t = v*v; out = ((t + a2)*t + b2) * in1
        [s0=-B, s1=a2, imm2=b2, in1 = g1 tensor]
    (the upper clamp min(u, B) is done by the preceding tensor_scalar)
    """
    import concourse.dve_ops as dve_ops
    from concourse.dve_spec import (
        Spec, Src0, Src1, C0, C1, C2, C3, sq, maxx,
        lower, _spill_c3_to_src1,
    )
    from concourse.dve_uop import DveOpSpec

    if "ATTN_TANH_P1" in dve_ops._SUB_OPCODE_FOR_NAME:
        by_name = {op.name: op for op in dve_ops.OPS}
        return by_name["ATTN_TANH_P1"], by_name["ATTN_TANH_P2"]

    def ref_p1(in0, in1, s0, s1, imm2):
        v = np.maximum(in0.astype(np.float32), np.float32(s0))
        t = v * v
        return (((in1.astype(np.float32) * t + np.float32(s1)) * t
                 + np.float32(imm2)) * v).astype(np.float32)

    def ref_p2(in0, in1, s0, s1, imm2):
        v = np.maximum(in0.astype(np.float32), np.float32(s0))
        t = v * v
        return (((t + np.float32(s1)) * t + np.float32(imm2))
                * in1.astype(np.float32)).astype(np.float32)

    _v1 = maxx(Src0, C0)
    _t1 = sq(_v1)
    body1 = _spill_c3_to_src1(((C3 * _t1 + C1) * _t1 + C2) * _v1)
    spec1 = Spec(body=body1, reference=ref_p1)

    _v2 = maxx(Src0, C0)
    _t2 = sq(_v2)
    body2 = ((_t2 + C1) * _t2 + C2) * Src1
    spec2 = Spec(body=body2, reference=ref_p2)

    ops = []
    for name, spec in [("ATTN_TANH_P1", spec1), ("ATTN_TANH_P2", spec2)]:
        opcode = dve_ops._CUSTOM_DVE_ROW_BASE + len(dve_ops.OPS)
        assert opcode < 0x20
        shas = {}
        for ver in ("v3", "v4"):
            s = DveOpSpec(name=name, opcode=opcode,
                          uops=lower(spec, ver=ver), rd1_en=True)
            shas[ver] = s.sha(ver)
        op = dve_ops.DveOp(name, spec, subdim=False, uops_sha=shas)
        dve_ops.OPS.append(op)
        dve_ops.CUSTOM_DVE_SPECS[name] = spec
        dve_ops._SUB_OPCODE_FOR_NAME[name] = opcode
        ops.append(op)
    return ops[0], ops[1]


def _build():
    P1, P2 = _register_dve_ops()
    xdt = mybir.dt.float16
    f32 = mybir.dt.float32

    nc = bacc.Bacc()
    xT = nc.declare_dram_parameter("xT", [D, S], xdt, isOutput=False)
    state_cols = nc.declare_dram_parameter("state_cols", [128, NB_D], f32,
                                           isOutput=False)
    w_blk = nc.declare_dram_parameter("w_blk", [NB_D, 128, 16 * 16], xdt,
                                      isOutput=False)
    out_ext = nc.declare_dram_parameter("out", [S], f32, isOutput=True)

    dma_q = {}

    with tile.TileContext(nc) as tc, ExitStack() as ctx:
        consts = ctx.enter_context(tc.tile_pool(name="consts", bufs=1))
        tpool = ctx.enter_context(tc.tile_pool(name="t", bufs=1))
        g1pool = ctx.enter_context(tc.tile_pool(name="g1", bufs=3))
        tailp = ctx.enter_context(tc.tile_pool(name="tail", bufs=1))
        psum = ctx.enter_context(tc.tile_pool(name="psum", bufs=2, space="PSUM"))

        # Dummy activation with no data deps: pulls the ACT_TABLE_LOAD
        # (~1.3 us, exp_and_others covers Tanh+Exp) into the preamble.
        warm = consts.tile([128, 1], f32)
        nc.vector.memset(warm, 0.0)
        nc.scalar.activation(out=warm, in_=warm,
                             func=mybir.ActivationFunctionType.Tanh)

        state_sb = consts.tile([128, NB_D], f32)
        nc.gpsimd.dma_start(out=state_sb, in_=state_cols[:])
        w_sb = consts.tile([128, NB_D, 256], xdt)
        nc.gpsimd.dma_start(out=w_sb, in_=w_blk[:].rearrange("i p c -> p i c"))

        lconst = consts.tile([128, 1], f32)
        nc.vector.memset(lconst, TANH_L)
        ones_sb = consts.tile([128, 16], f32)
        nc.vector.memset(ones_sb, 1.0)
        sums_sb = consts.tile([128, 1], f32)
        nc.vector.memset(sums_sb, 0.0)

        # piece tiles (resident; x for a DVE piece is overwritten in place:
        # ts: x <- min(x + state, B); p2: x <- tanh_approx)
        tiles = {}
        for k, (i, c0, w, q) in enumerate(PIECE_SCHED):
            t_t = tpool.tile([128, w], xdt, tag=f"t{k}", name=f"t{k}")
            tiles[k] = t_t

        # DMA issues, alternating between the two DMA rings
        for k, (i, c0, w, q) in enumerate(PIECE_SCHED):
            eng = nc.sync if q == 0 else nc.gpsimd
            eng.dma_start(
                out=tiles[k], in_=xT[:][128 * i:128 * (i + 1), c0:c0 + w],
            )

        # compute: ACT tanh for ACT pieces; DVE ts + p1 + p2 for DVE pieces
        for k, (i, c0, w, q) in enumerate(PIECE_SCHED):
            t_t = tiles[k]
            if (i, c0) in DVE_PIECES:
                nc.vector.tensor_scalar(
                    out=t_t, in0=t_t,
                    scalar1=state_sb[:, i:i + 1], scalar2=TANH_B,
                    op0=mybir.AluOpType.add, op1=mybir.AluOpType.min,
                )
                g1 = g1pool.tile([128, w], xdt, tag="g1", name=f"g1_{k}")
                nc.vector._custom_dve(
                    P1, out=g1, in0=t_t, in1=lconst,
                    s0=-TANH_B, s1=TANH_A1, imm2=TANH_B1,
                )
                nc.vector._custom_dve(
                    P2, out=t_t, in0=t_t, in1=g1,
                    s0=-TANH_B, s1=TANH_A2, imm2=TANH_B2,
                )
            else:
                nc.scalar.activation(
                    out=t_t, in_=t_t,
                    func=mybir.ActivationFunctionType.Tanh,
                    bias=state_sb[:, i:i + 1], scale=1.0,
                )

        energy_ps = psum.tile([16, 512], f32)
        n_mm = 0
        n_total = sum(w // 512 for (_, _, w, _) in PIECE_SCHED)
        for k, (i, c0, w, q) in enumerate(PIECE_SCHED):
            t_t = tiles[k]
            for c in range(c0 // 512, (c0 + w) // 512):
                off = 512 * c - c0
                n_mm += 1
                nc.tensor.matmul(
                    energy_ps[:],
                    lhsT=w_sb[:, i, 16 * c:16 * (c + 1)],
                    rhs=t_t[:, off:off + 512],
                    start=(n_mm == 1),
                    stop=(n_mm == n_total),
                )

        # softmax tail (softmax max-subtraction is skipped: |energy| <= ||w||_1
        # ~ 26, exp is safely in fp32 range; the bias b never affects softmax)
        p_sb = tailp.tile([16, 512], f32)
        nc.scalar.activation(
            out=p_sb, in_=energy_ps[:],
            func=mybir.ActivationFunctionType.Exp,
            bias=0.0, scale=1.0,
            accum_out=sums_sb[0:16, :],
        )
        sum_ps = psum.tile([16, 1], f32)
        nc.tensor.matmul(sum_ps[:], lhsT=ones_sb, rhs=sums_sb,
                         start=True, stop=True)
        inv_sb = tailp.tile([16, 1], f32)
        nc.vector.reciprocal(out=inv_sb, in_=sum_ps[:])
        out_sb = tailp.tile([16, 512], f32)
        nc.vector.tensor_scalar_mul(out=out_sb, in0=p_sb, scalar1=inv_sb)
        nc.sync.dma_start(
            out=out_ext[:].rearrange("(p f) -> p f", p=16), in_=out_sb,
        )

    nc.finalize()
    return nc


def _get_nc():
    if "nc" not in _compiled:
        _compiled["nc"] = _build()
    return _compiled["nc"]


def kernel(input_sec, state, w, b=None, **_unused):
    np_xdt = np.float16
    nc = _get_nc()

    # host-side layout prep (single-pass strided read + cast + pack)
    xT_all = np.asarray(input_sec).transpose(0, 2, 1).astype(np_xdt)  # [B, D, S]
    state_cols_all = np.ascontiguousarray(
        np.asarray(state, np.float32).reshape(B, NB_D, 128).transpose(0, 2, 1)
    )                                                          # [B, 128, NB_D]
    w_grid = np.asarray(w, np.float32).reshape(NB_D, 128)
    w_blk = np.zeros((NB_D, 128, 16, 16), np.float32)
    for j in range(16):
        w_blk[:, :, j, j] = w_grid
    w_blk = w_blk.reshape(NB_D, 128, 256).astype(np_xdt)

    in_maps = [
        {
            "xT": xT_all[c],
            "state_cols": state_cols_all[c],
            "w_blk": w_blk,
        }
        for c in range(B)
    ]
    trace = bool(int(os.environ.get("ATTN_KERNEL_TRACE", "0")))
    res = run_bass_kernel_spmd(nc, in_maps, core_ids=list(range(B)),
                               trace=trace)
    global last_result
    last_result = res
    out = np.stack([res.results[c]["out"] for c in range(B)], axis=0)
    return out.astype(np.float32)


# revision 6
# speedup vs baseline: 4.5999x; 1.1706x over previous
"""Trainium2 Bass kernel for attention-score softmax.

Computes, for input_sec [B=8, S=8192, D=1024], state [B, D], w [D], b [1]:
    energy[b, s] = dot(tanh(input_sec[b, s, :] + state[b, :]), w) + b
    out[b, :]    = softmax(energy[b, :], axis=-1)

Sharding: data-parallel over batch, one batch element per NeuronCore (8 cores).

Per-core dataflow on transposed input xT [D, S] in fp16 (host-side cast —
halves DMA traffic; this kernel is memory-bound):
  - x arrives in column pieces via two parallel HWDGE rings (sync + scalar
    queues) so per-piece completion stalls overlap across rings.
  - tanh is split across TWO engines to break the ScalarE 1-elem/cycle wall:
      * ACT (ScalarE): 6 of 8 d-blocks, exact tanh, bias=state fused.
      * DVE (VectorE): 2 d-blocks via a deg-9 odd polynomial on the clamped
        input (max err 4.8e-3), evaluated in 2 custom 8-stage DVE ops;
        the bias-add + upper clamp runs as a tensor_scalar on GpSimd (Pool),
        which is otherwise idle.  f(u) = v*(L*t^2+a1*t+b1)*(t^2+a2*t+b2),
        v = clamp(u, +-B), t = v*v.
  - TensorE: energy = w . t accumulated over all pieces into one PSUM tile
    [16, 512]; sequence chunk j lands on PSUM partition j via block-diagonal
    weight columns.
  - ScalarE: p = exp(energy) with fused row sums; TensorE ones-matmul reduces
    and broadcasts the total; VectorE reciprocal + scale; DMA out.
"""

import os
from contextlib import ExitStack

import numpy as np

import concourse.bacc as bacc
import concourse.tile as tile
from concourse import mybir
from concourse.bass_utils import run_bass_kernel_spmd

B, S, D = 8, 8192, 1024
NB_D = D // 128          # 8 d-blocks
N_CHUNK = S // 512       # 16 sequence chunks of 512

# --- DVE tanh approximation constants (fit: max err 4.8e-3 on |u|<=9) -------
TANH_B = 2.848135051824187       # clamp bound
TANH_L = 0.0002268581482379952   # leading coeff of q1
TANH_A1 = -0.004724477388143275
TANH_B1 = 0.028162570473750825
TANH_A2 = -3.3548299414719067
TANH_B2 = 34.733766917451845

# (block, piece col range) pairs consumed by the DVE path; everything else ACT.
DVE_PIECES = {(2, c) for c in range(0, 8192, 2048)} | \
             {(5, c) for c in range(0, 8192, 2048)}   # 2 of 8 d-blocks

# Global piece schedule: (block, col0, width, queue) in issue order.
# All x pieces ride ONE HWDGE ring (sync / qSPDynamicHW): a single
# sequential stream keeps the SDMA engines at the HBM ceiling — a second
# concurrent ring was measured to hurt HBM locality (~210 GB/s aggregate).
# The stream is arrival-paced: during the slow DMA ramp, only small
# ACT-consumed pieces are scheduled so ScalarE never idles; DVE pieces
# (whose 3-pass chain has slack) slot in from ~20us on.  The final pieces
# are small to shorten the softmax tail.
PIECE_SCHED = [
    (0, 0,    1024, 0),
    (1, 0,    1024, 0),
    (0, 1024, 1024, 0),
    (1, 1024, 1024, 0),
    (2, 0,    2048, 0),
    (0, 2048, 2048, 0),
    (3, 0,    2048, 0),
    (2, 2048, 2048, 0),
    (1, 2048, 2048, 0),
    (0, 4096, 4096, 0),
    (2, 4096, 2048, 0),
    (3, 2048, 2048, 0),
    (1, 4096, 4096, 0),
    (2, 6144, 2048, 0),
    (3, 4096, 4096, 0),
    (5, 0,    2048, 0),
    (4, 0,    4096, 0),
    (5, 2048, 2048, 0),
    (6, 0,    4096, 0),
    (5, 4096, 2048, 0),
    (4, 4096, 4096, 0),
    (5, 6144, 2048, 0),
    (6, 4096, 4096, 0),
    (7, 0,    4096, 0),
    (7, 4096, 2048, 0),
    (7, 6144, 1536, 0),
    (7, 7680, 512,  0),
]

_compiled = {}
last_result = None  # BassKernelResults of the most recent run (for test harness)


# --- custom DVE op registration --------------------------------------------
def _register_dve_ops():
    """Register the two custom DVE tanh ops in concourse.dve_ops (idempotent).

    P1: v = max(in0, s0); t = v*v; out = ((L*t + a1)*t + b1) * v
        [s0=-B, C3(in1,[P,1])=L, s1=a1, imm2=b1]
    P2: v = max(in0, s0); t = v*v; out = ((t + a2)*t + b2) * in1
        [s0=-B, s1=a2, imm2=b2, in1 = g1 tensor]
    (the upper clamp min(u, B) is done by the preceding tensor_scalar)
    """
    import concourse.dve_ops as dve_ops
    from concourse.dve_spec import (
        Spec, Src0, Src1, C0, C1, C2, C3, sq, maxx,
        lower, _spill_c3_to_src1,
    )
    from concourse.dve_uop import DveOpSpec

    if "ATTN_TANH_P1" in dve_ops._SUB_OPCODE_FOR_NAME:
        by_name = {op.name: op for op in dve_ops.OPS}
        return by_name["ATTN_TANH_P1"], by_name["ATTN_TANH_P2"]

    def ref_p1(in0, in1, s0, s1, imm2):
        v = np.maximum(in0.astype(np.float32), np.float32(s0))
        t = v * v
        return (((in1.astype(np.float32) * t + np.float32(s1)) * t
                 + np.float32(imm2)) * v).astype(np.float32)

    def ref_p2(in0, in1, s0, s1, imm2):
        v = np.maximum(in0.astype(np.float32), np.float32(s0))
        t = v * v
        return (((t + np.float32(s1)) * t + np.float32(imm2))
                * in1.astype(np.float32)).astype(np.float32)

    _v1 = maxx(Src0, C0)
    _t1 = sq(_v1)
    body1 = _spill_c3_to_src1(((C3 * _t1 + C1) * _t1 + C2) * _v1)
    spec1 = Spec(body=body1, reference=ref_p1)

    _v2 = maxx(Src0, C0)
    _t2 = sq(_v2)
    body2 = ((_t2 + C1) * _t2 + C2) * Src1
    spec2 = Spec(body=body2, reference=ref_p2)

    ops = []
    for name, spec in [("ATTN_TANH_P1", spec1), ("ATTN_TANH_P2", spec2)]:
        opcode = dve_ops._CUSTOM_DVE_ROW_BASE + len(dve_ops.OPS)
        assert opcode < 0x20
        shas = {}
        for ver in ("v3", "v4"):
            s = DveOpSpec(name=name, opcode=opcode,
                          uops=lower(spec, ver=ver), rd1_en=True)
            shas[ver] = s.sha(ver)
        op = dve_ops.DveOp(name, spec, subdim=False, uops_sha=shas)
        dve_ops.OPS.append(op)
        dve_ops.CUSTOM_DVE_SPECS[name] = spec
        dve_ops._SUB_OPCODE_FOR_NAME[name] = opcode
        ops.append(op)
    return ops[0], ops[1]


def _build():
    P1, P2 = _register_dve_ops()
    xdt = mybir.dt.float16
    f32 = mybir.dt.float32

    nc = bacc.Bacc()
    xT = nc.declare_dram_parameter("xT", [D, S], xdt, isOutput=False)
    state_cols = nc.declare_dram_parameter("state_cols", [128, NB_D], f32,
                                           isOutput=False)
    w_blk = nc.declare_dram_parameter("w_blk", [NB_D, 128, 16 * 16], xdt,
                                      isOutput=False)
    out_ext = nc.declare_dram_parameter("out", [S], f32, isOutput=True)

    dma_q = {}

    with tile.TileContext(nc) as tc, ExitStack() as ctx:
        consts = ctx.enter_context(tc.tile_pool(name="consts", bufs=1))
        tpool = ctx.enter_context(tc.tile_pool(name="t", bufs=1))
        g1pool = ctx.enter_context(tc.tile_pool(name="g1", bufs=3))
        tailp = ctx.enter_context(tc.tile_pool(name="tail", bufs=1))
        psum = ctx.enter_context(tc.tile_pool(name="psum", bufs=2, space="PSUM"))

        # Dummy activation with no data deps: pulls the ACT_TABLE_LOAD
        # (~1.3 us, exp_and_others covers Tanh+Exp) into the preamble.
        warm = consts.tile([128, 1], f32)
        nc.vector.memset(warm, 0.0)
        nc.scalar.activation(out=warm, in_=warm,
                             func=mybir.ActivationFunctionType.Tanh)

        state_sb = consts.tile([128, NB_D], f32)
        nc.gpsimd.dma_start(out=state_sb, in_=state_cols[:])
        w_sb = consts.tile([128, NB_D, 256], xdt)
        nc.gpsimd.dma_start(out=w_sb, in_=w_blk[:].rearrange("i p c -> p i c"))

        lconst = consts.tile([128, 1], f32)
        nc.vector.memset(lconst, TANH_L)
        ones_sb = consts.tile([128, 16], f32)
        nc.vector.memset(ones_sb, 1.0)
        sums_sb = consts.tile([128, 1], f32)
        nc.vector.memset(sums_sb, 0.0)

        # piece tiles (resident; x for a DVE piece is overwritten in place:
        # ts: x <- min(x + state, B); p2: x <- tanh_approx)
        tiles = {}
        for k, (i, c0, w, q) in enumerate(PIECE_SCHED):
            t_t = tpool.tile([128, w], xdt, tag=f"t{k}", name=f"t{k}")
            tiles[k] = t_t

        # DMA issues, alternating between the two DMA rings
        for k, (i, c0, w, q) in enumerate(PIECE_SCHED):
            eng = nc.sync if q == 0 else nc.gpsimd
            eng.dma_start(
                out=tiles[k], in_=xT[:][128 * i:128 * (i + 1), c0:c0 + w],
            )

        # compute: ACT tanh for ACT pieces; DVE ts + p1 + p2 for DVE pieces
        for k, (i, c0, w, q) in enumerate(PIECE_SCHED):
            t_t = tiles[k]
            if (i, c0) in DVE_PIECES:
                nc.vector.tensor_scalar(
                    out=t_t, in0=t_t,
                    scalar1=state_sb[:, i:i + 1], scalar2=TANH_B,
                    op0=mybir.AluOpType.add, op1=mybir.AluOpType.min,
                )
                g1 = g1pool.tile([128, w], xdt, tag="g1", name=f"g1_{k}")
                nc.vector._custom_dve(
                    P1, out=g1, in0=t_t, in1=lconst,
                    s0=-TANH_B, s1=TANH_A1, imm2=TANH_B1,
                )
                nc.vector._custom_dve(
                    P2, out=t_t, in0=t_t, in1=g1,
                    s0=-TANH_B, s1=TANH_A2, imm2=TANH_B2,
                )
            else:
                nc.scalar.activation(
                    out=t_t, in_=t_t,
                    func=mybir.ActivationFunctionType.Tanh,
                    bias=state_sb[:, i:i + 1], scale=1.0,
                )

        energy_ps = psum.tile([16, 512], f32)
        n_mm = 0
        n_total = sum(w // 512 for (_, _, w, _) in PIECE_SCHED)
        for k, (i, c0, w, q) in enumerate(PIECE_SCHED):
            t_t = tiles[k]
            for c in range(c0 // 512, (c0 + w) // 512):
                off = 512 * c - c0
                n_mm += 1
                nc.tensor.matmul(
                    energy_ps[:],
                    lhsT=w_sb[:, i, 16 * c:16 * (c + 1)],
                    rhs=t_t[:, off:off + 512],
                    start=(n_mm == 1),
                    stop=(n_mm == n_total),
                )

        # softmax tail (softmax max-subtraction is skipped: |energy| <= ||w||_1
        # ~ 26, exp is safely in fp32 range; the bias b never affects softmax)
        p_sb = tailp.tile([16, 512], f32)
        nc.scalar.activation(
            out=p_sb, in_=energy_ps[:],
            func=mybir.ActivationFunctionType.Exp,
            bias=0.0, scale=1.0,
            accum_out=sums_sb[0:16, :],
        )
        sum_ps = psum.tile([16, 1], f32)
        nc.tensor.matmul(sum_ps[:], lhsT=ones_sb, rhs=sums_sb,
                         start=True, stop=True)
        inv_sb = tailp.tile([16, 1], f32)
        nc.vector.reciprocal(out=inv_sb, in_=sum_ps[:])
        out_sb = tailp.tile([16, 512], f32)
        nc.vector.tensor_scalar_mul(out=out_sb, in0=p_sb, scalar1=inv_sb)
        nc.sync.dma_start(
            out=out_ext[:].rearrange("(p f) -> p f", p=16), in_=out_sb,
        )

    nc.finalize()
    return nc


def _get_nc():
    if "nc" not in _compiled:
        _compiled["nc"] = _build()
    return _compiled["nc"]


def kernel(input_sec, state, w, b=None, **_unused):
    np_xdt = np.float16
    nc = _get_nc()

    # host-side layout prep (single-pass strided read + cast + pack)
    xT_all = np.asarray(input_sec).transpose(0, 2, 1).astype(np_xdt)  # [B, D, S]
    state_cols_all = np.ascontiguousarray(
        np.asarray(state, np.float32).reshape(B, NB_D, 128).transpose(0, 2, 1)
    )                                                          # [B, 128, NB_D]
    w_grid = np.asarray(w, np.float32).reshape(NB_D, 128)
    w_blk = np.zeros((NB_D, 128, 16, 16), np.float32)
    for j in range(16):
        w_blk[:, :, j, j] = w_grid
    w_blk = w_blk.reshape(NB_D, 128, 256).astype(np_xdt)

    in_maps = [
        {
            "xT": xT_all[c],
            "state_cols": state_cols_all[c],
            "w_blk": w_blk,
        }
        for c in range(B)
    ]
    trace = bool(int(os.environ.get("ATTN_KERNEL_TRACE", "0")))
    res = run_bass_kernel_spmd(nc, in_maps, core_ids=list(range(B)),
                               trace=trace)
    global last_result
    last_result = res
    out = np.stack([res.results[c]["out"] for c in range(B)], axis=0)
    return out.astype(np.float32)
